# revision 1
# baseline (speedup 1.0000x reference)
"""Trainium2 Bass kernel for the contextual-bandit routing module.

Math (collapsed form of the reference network; biases kept general):
  ctx[b]      = concat(cemb[x[b,0]], cemb[x[b,1]])                 # [2D=128]
  P_a         = W2[a] @ W1[a]            c_a = W2[a]@b1[a] + b2[a] # [D,2D], [D]
  preds[b,a]  = P_a @ ctx[b] + c_a
  Q_a         = Wp @ P_a + Wc            d_a = Wp @ c_a + br1      # [H,2D], [H]
  z[b,a]      = relu(Q_a @ ctx[b] + d_a)
  rewards     = Wr2 . z[b,a]  (+br2, irrelevant for argmin)
  out r[b]    = preds[b, argmin_a rewards]
  out wemb[b] = wemb_table[y[b]]

Sharding: data-parallel over batch across 8 cores; weights replicated.
The rewards path stays fp32 end-to-end (bf16 flips ~131 argmins -> 8.5% err).
"""

import sys

sys.path.insert(0, "/opt/trn_rl_repo")

from contextlib import ExitStack

import numpy as np

import concourse.bass as bass
import concourse.bacc as bacc
import concourse.mybir as mybir
import concourse.tile as tile
from concourse.bass_utils import run_bass_kernel_spmd

F32 = mybir.dt.float32
I32 = mybir.dt.int32
U8 = mybir.dt.uint8
U32 = mybir.dt.uint32
F32R = mybir.dt.float32r

NCORES = 8
B, A, D, H, V = 32768, 32, 64, 128, 50000
D2 = 2 * D  # 128
BC = B // NCORES  # 4096 samples per core
TILE = 512  # samples per tile
NT = BC // TILE  # 8 tiles
CH = TILE // 128  # 4 chunks of 128 samples
NPAIR = A // 2  # 16 arm pairs

_CACHE = {}


def _build_program(loop_reps=1, upto=7, preds_f32r=True, preds_copy='split'):
    nc = bacc.Bacc(
        "TRN2", target_bir_lowering=False, debug=False, num_devices=NCORES
    )

    cemb = nc.dram_tensor("cemb", [V, D], F32, kind="ExternalInput").ap()
    wembt = nc.dram_tensor("wembt", [V, D], F32, kind="ExternalInput").ap()
    xidx0_d = nc.dram_tensor("xidx0", [128, NT * CH], I32, kind="ExternalInput").ap()
    xidx1_d = nc.dram_tensor("xidx1", [128, NT * CH], I32, kind="ExternalInput").ap()
    yidx_d = nc.dram_tensor("yidx", [128, NT * CH], I32, kind="ExternalInput").ap()
    PT_d = nc.dram_tensor("PT", [128, NPAIR * 128], F32R, kind="ExternalInput").ap()
    QT_d = nc.dram_tensor("QT", [128, A * 128], F32, kind="ExternalInput").ap()
    dmat_d = nc.dram_tensor("dmat", [128, A], F32, kind="ExternalInput").ap()
    negw_d = nc.dram_tensor("negw", [128, 1], F32, kind="ExternalInput").ap()
    cmat_d = nc.dram_tensor("cmat", [A, D], F32, kind="ExternalInput").ap()
    armv_d = nc.dram_tensor("armv", [128, NPAIR], F32, kind="ExternalInput").ap()
    iotaA_d = nc.dram_tensor("iotaA", [128, A], F32, kind="ExternalInput").ap()
    iotaC_d = nc.dram_tensor("iotaC", [A, 1], F32, kind="ExternalInput").ap()
    ident_d = nc.dram_tensor("ident", [128, 128], F32, kind="ExternalInput").ap()

    r_out = nc.dram_tensor("r_out", [BC, D], F32, kind="ExternalOutput").ap()
    w_out = nc.dram_tensor("w_out", [BC, D], F32, kind="ExternalOutput").ap()

    RELU = mybir.ActivationFunctionType.Relu
    EQ = mybir.AluOpType.is_equal
    ADD = mybir.AluOpType.add

    with tile.TileContext(nc) as tc, ExitStack() as ctx:
        const = ctx.enter_context(tc.tile_pool(name="const", bufs=1))
        gpool = ctx.enter_context(tc.tile_pool(name="g", bufs=3))
        cpool = ctx.enter_context(tc.tile_pool(name="ctxp", bufs=2))
        zpool = ctx.enter_context(tc.tile_pool(name="zr", bufs=4))
        mpool = ctx.enter_context(tc.tile_pool(name="mask", bufs=2))
        spool = ctx.enter_context(tc.tile_pool(name="small", bufs=2))
        opool = ctx.enter_context(tc.tile_pool(name="outs", bufs=2))
        ppool = ctx.enter_context(tc.tile_pool(name="predsp", bufs=2))
        ps_z = ctx.enter_context(tc.tile_pool(name="psz", bufs=2, space="PSUM"))
        ps_p = ctx.enter_context(tc.tile_pool(name="psp", bufs=2, space="PSUM"))
        ps_nr = ctx.enter_context(tc.tile_pool(name="psnr", bufs=2, space="PSUM"))
        ps_t = ctx.enter_context(tc.tile_pool(name="pst", bufs=2, space="PSUM"))

        def load_const(name, dram_ap, shape, dtype=F32):
            t = const.tile(shape, dtype, tag=name)
            nc.sync.dma_start(out=t[:], in_=dram_ap)
            return t

        PT = load_const("PT", PT_d, [128, NPAIR * 128], F32R)
        QT = load_const("QT", QT_d, [128, A * 128])
        dmat = load_const("dmat", dmat_d, [128, A])
        negw = load_const("negw", negw_d, [128, 1])
        cmat = load_const("cmat", cmat_d, [A, D])
        armv = load_const("armv", armv_d, [128, NPAIR])
        iotaA = load_const("iotaA", iotaA_d, [128, A])
        iotaC = load_const("iotaC", iotaC_d, [A, 1])
        ident = load_const("ident", ident_d, [128, 128])
        xs0 = load_const("xs0", xidx0_d, [128, NT * CH], I32)
        xs1 = load_const("xs1", xidx1_d, [128, NT * CH], I32)
        ys = load_const("ys", yidx_d, [128, NT * CH], I32)

        def produce(t):
            """Gathers, ctxT, z+negrew, preds — the PE-dense half."""
            st = {}
            gx0 = gpool.tile([128, CH, D], F32, tag="gx0")
            gx1 = gpool.tile([128, CH, D], F32, tag="gx1")
            gy = gpool.tile([128, CH, D], F32, tag="gy")
            for j in range(CH):
                col = t * CH + j
                nc.gpsimd.indirect_dma_start(
                    out=gx0[:, j, :], out_offset=None, in_=cemb,
                    in_offset=bass.IndirectOffsetOnAxis(
                        ap=xs0[:, col : col + 1], axis=0),
                )
                nc.gpsimd.indirect_dma_start(
                    out=gx1[:, j, :], out_offset=None, in_=cemb,
                    in_offset=bass.IndirectOffsetOnAxis(
                        ap=xs1[:, col : col + 1], axis=0),
                )
                nc.gpsimd.indirect_dma_start(
                    out=gy[:, j, :], out_offset=None, in_=wembt,
                    in_offset=bass.IndirectOffsetOnAxis(
                        ap=ys[:, col : col + 1], axis=0),
                )
            w_slice = w_out[t * TILE : (t + 1) * TILE, :].rearrange(
                "(j p) d -> p j d", p=128
            )
            nc.sync.dma_start(out=w_slice, in_=gy[:])
            if upto < 2:
                return st

            ctxT = cpool.tile([128, TILE], F32, tag="ctxT")
            ctp0 = ps_t.tile([64, TILE], F32, tag="misc")
            ctp1 = ps_t.tile([64, TILE], F32, tag="misc")
            for j in range(CH):
                sl = slice(j * 128, (j + 1) * 128)
                nc.tensor.transpose(
                    out=ctp0[:, sl], in_=gx0[:, j, :], identity=ident[:]
                )
                nc.tensor.transpose(
                    out=ctp1[:, sl], in_=gx1[:, j, :], identity=ident[:]
                )
            nc.vector.tensor_copy(out=ctxT[0:64, :], in_=ctp0[:])
            nc.vector.tensor_copy(out=ctxT[64:128, :], in_=ctp1[:])
            st["ctxT"] = ctxT
            ctxTr = cpool.tile([128, TILE], F32R, tag="ctxTr")
            nc.vector.tensor_copy(out=ctxTr[:], in_=ctxT[:])
            st["ctxTr"] = ctxTr
            if upto < 3:
                return st

            nr = ps_nr.tile([128, CH, A], F32, tag="nr")
            st["nr"] = nr
            zr_tiles = {}
            for a in range(A):
                zps = ps_z.tile([128, TILE], F32, tag="z")
                nc.tensor.matmul(
                    out=zps[:], lhsT=QT[:, a * 128 : (a + 1) * 128],
                    rhs=ctxT[:], start=True, stop=True,
                )
                zr = zpool.tile([128, TILE], F32, tag="zr")
                nc.scalar.activation(
                    out=zr[:], in_=zps[:], func=RELU,
                    bias=dmat[:, a : a + 1], scale=1.0,
                )
                zr_tiles[a] = zr
                if upto >= 4 and a > 0:
                    zp = zr_tiles.pop(a - 1)
                    for c in range(CH):
                        nc.tensor.matmul(
                            out=nr[:, c, a - 1 : a],
                            lhsT=zp[:, c * 128 : (c + 1) * 128],
                            rhs=negw[:], start=True, stop=True,
                        )
            if upto >= 4:
                zp = zr_tiles.pop(A - 1)
                for c in range(CH):
                    nc.tensor.matmul(
                        out=nr[:, c, A - 1 : A],
                        lhsT=zp[:, c * 128 : (c + 1) * 128],
                        rhs=negw[:], start=True, stop=True,
                    )
            if upto < 5:
                return st

            preds_sb = ppool.tile([128, NPAIR, TILE], F32, tag="preds")
            st["preds"] = preds_sb
            for j in range(NPAIR):
                pps = ps_p.tile([128, TILE], F32, tag="pp")
                if preds_f32r:
                    nc.tensor.matmul(
                        out=pps[:], lhsT=PT[:, j * 128 : (j + 1) * 128],
                        rhs=st["ctxTr"][:], start=True, stop=True,
                    )
                else:
                    nc.tensor.matmul(
                        out=pps[:],
                        lhsT=PT[:, j * 128 : (j + 1) * 128].bitcast(F32),
                        rhs=st["ctxT"][:], start=True, stop=True,
                    )
                if preds_copy == "act" or (preds_copy == "split" and j % 2 == 0):
                    nc.scalar.copy(out=preds_sb[:, j, :], in_=pps[:])
                else:
                    nc.vector.tensor_copy(out=preds_sb[:, j, :], in_=pps[:])
            return st

        def route(t, st):
            """Argmin + one-hot S + masks + selected bias (DVE/PE light)."""
            if upto < 6 or "nr" not in st:
                return
            nr = st["nr"]
            S = spool.tile([A, TILE], F32, tag="S")
            for c in range(CH):
                nrs = spool.tile([128, A], F32, tag="nrs")
                nc.vector.tensor_copy(out=nrs[:], in_=nr[:, c, :])
                mx8 = spool.tile([128, 8], F32, tag="mx8")
                ix8 = spool.tile([128, 8], U32, tag="ix8")
                nc.vector.max(out=mx8[:], in_=nrs[:])
                nc.vector.max_index(out=ix8[:], in_max=mx8[:], in_values=nrs[:])
                ixf = spool.tile([128, 1], F32, tag="ixf")
                nc.vector.tensor_copy(out=ixf[:], in_=ix8[:, 0:1])
                oh = spool.tile([128, A], F32, tag="oh")
                nc.vector.tensor_scalar(
                    out=oh[:], in0=iotaA[:], scalar1=ixf[:], scalar2=None, op0=EQ
                )
                Sps = ps_t.tile([A, 128], F32, tag="misc")
                nc.tensor.transpose(out=Sps[:], in_=oh[:], identity=ident[:])
                nc.scalar.copy(out=S[:, c * 128 : (c + 1) * 128], in_=Sps[:])

            ixTps = ps_t.tile([1, TILE], F32, tag="misc")
            nc.tensor.matmul(
                out=ixTps[:], lhsT=iotaC[:], rhs=S[:], start=True, stop=True
            )
            ixT = spool.tile([1, TILE], F32, tag="ixT")
            nc.vector.tensor_copy(out=ixT[:], in_=ixTps[:])
            ixB = mpool.tile([128, TILE], F32, tag="ixB")
            nc.gpsimd.partition_broadcast(ixB[:], ixT[:], channels=128)
            masks = mpool.tile([128, NPAIR, TILE], U8, tag="masks")
            st["masks"] = masks
            for j in range(NPAIR):
                nc.vector.tensor_scalar(
                    out=masks[:, j, :], in0=ixB[:],
                    scalar1=armv[:, j : j + 1], scalar2=None, op0=EQ,
                )
            cselp = ps_t.tile([D, TILE], F32, tag="misc")
            nc.tensor.matmul(
                out=cselp[:], lhsT=cmat[:], rhs=S[:], start=True, stop=True
            )
            csel = opool.tile([D, TILE], F32, tag="csel")
            nc.scalar.copy(out=csel[:], in_=cselp[:])
            st["csel"] = csel

        def finish(t, st):
            """Predicated select + merge + transpose-out + store."""
            if upto < 7 or "masks" not in st:
                return
            masks = st["masks"]
            preds_sb = st["preds"]
            csel = st["csel"]
            rsel2 = opool.tile([128, TILE], F32, tag="rsel2")
            nc.vector.memset(rsel2[:], 0.0)
            for j in range(NPAIR):
                nc.vector.copy_predicated(
                    out=rsel2[:], mask=masks[:, j, :], data=preds_sb[:, j, :]
                )
            rte = opool.tile([D, TILE], F32, tag="rte")
            nc.vector.tensor_tensor(
                out=rte[:], in0=rsel2[0:64, :], in1=csel[:], op=ADD
            )
            for c in range(CH):
                sl = slice(c * 128, (c + 1) * 128)
                tpe = ps_t.tile([128, D], F32, tag="misc")
                nc.tensor.matmul(
                    out=tpe[:], lhsT=rte[:, sl], rhs=ident[0:64, 0:64],
                    is_transpose=True, start=True, stop=True,
                )
                tpo = ps_t.tile([128, D], F32, tag="misc")
                nc.tensor.matmul(
                    out=tpo[:], lhsT=rsel2[64:128, sl],
                    rhs=ident[64:128, 64:128], start=True, stop=True,
                )
                tse = opool.tile([128, D], F32, tag="tse")
                nc.scalar.copy(out=tse[:], in_=tpe[:])
                rts = opool.tile([128, D], F32, tag="rts")
                nc.vector.tensor_tensor(
                    out=rts[:], in0=tse[:], in1=tpo[:], op=ADD
                )
                base = t * TILE + c * 128
                nc.sync.dma_start(out=r_out[base : base + 128, :], in_=rts[:])

        if loop_reps > 1:
            loop_cm = tc.For_i(0, loop_reps, 1)
            loop_cm.__enter__()

        # software pipeline with 1-tile skew: tile t-1's routing/select runs
        # on DVE/ACT while tile t's PE-dense produce phase streams.
        prev = None
        for t in range(NT):
            if prev is not None:
                route(t - 1, prev)
            st = produce(t)
            if prev is not None:
                finish(t - 1, prev)
            prev = st
        route(NT - 1, prev)
        finish(NT - 1, prev)

        if loop_reps > 1:
            loop_cm.__exit__(None, None, None)

    nc.compile()
    return nc


def _host_prep(x, y, cemb_table, wemb_table, W1, b1, W2, b2, Wr1, br1, Wr2, br2):
    """Collapse the per-arm networks (fp64 for accuracy, cast to fp32)."""
    W1_ = W1.astype(np.float64)
    W2_ = W2.astype(np.float64)
    b1_ = b1.astype(np.float64)
    b2_ = b2.astype(np.float64)
    Wc = Wr1[:, :D2].astype(np.float64)  # [H, 2D]
    Wp = Wr1[:, D2:].astype(np.float64)  # [H, D]
    br1_ = br1.astype(np.float64)

    P = np.einsum("adh,ahi->adi", W2_, W1_)  # [A, D, 2D]
    c = np.einsum("adh,ah->ad", W2_, b1_) + b2_  # [A, D]
    Q = np.einsum("hd,adi->ahi", Wp, P) + Wc[None, :, :]  # [A, H, 2D]
    dv = np.einsum("hd,ad->ah", Wp, c) + br1_[None, :]  # [A, H]

    PT = np.concatenate(
        [
            np.concatenate([P[2 * j].T, P[2 * j + 1].T], axis=1)
            for j in range(NPAIR)
        ],
        axis=1,
    ).astype(np.float32)  # [2D, NPAIR*128]
    QT = np.concatenate([Q[a].T for a in range(A)], axis=1).astype(
        np.float32
    )  # [2D, A*128]
    dmat = dv.T.astype(np.float32)  # [H, A]
    negw = (-Wr2.astype(np.float64))[:, None].astype(np.float32)  # [H, 1]
    cmat = c.astype(np.float32)  # [A, D]

    armv = np.empty((128, NPAIR), np.float32)
    for j in range(NPAIR):
        armv[:64, j] = 2 * j
        armv[64:, j] = 2 * j + 1
    iotaA = np.tile(np.arange(A, dtype=np.float32)[None, :], (128, 1))
    iotaC = np.arange(A, dtype=np.float32)[:, None]
    ident = np.eye(128, dtype=np.float32)

    x32 = np.ascontiguousarray(np.asarray(x).astype(np.int32))
    y32 = np.ascontiguousarray(np.asarray(y).astype(np.int32))

    def idx_layout(v):  # [BC] -> [128, NT*CH] with col t*CH+j, row p
        return np.ascontiguousarray(
            v.reshape(NT, CH, 128).transpose(2, 0, 1).reshape(128, NT * CH)
        )

    shared = dict(
        cemb=np.ascontiguousarray(cemb_table.astype(np.float32)),
        wembt=np.ascontiguousarray(wemb_table.astype(np.float32)),
        PT=np.ascontiguousarray(PT),
        QT=np.ascontiguousarray(QT),
        dmat=np.ascontiguousarray(dmat),
        negw=np.ascontiguousarray(negw),
        cmat=np.ascontiguousarray(cmat),
        armv=armv,
        iotaA=np.ascontiguousarray(iotaA),
        iotaC=iotaC,
        ident=ident,
    )
    in_maps = []
    for k in range(NCORES):
        lo, hi = k * BC, (k + 1) * BC
        m = dict(shared)
        m["xidx0"] = idx_layout(x32[lo:hi, 0])
        m["xidx1"] = idx_layout(x32[lo:hi, 1])
        m["yidx"] = idx_layout(y32[lo:hi])
        in_maps.append(m)
    return in_maps


def _get_nc():
    if "nc" not in _CACHE:
        _CACHE["nc"] = _build_program()
    return _CACHE["nc"]


def run(inputs, trace=False, **kw):
    """Build + execute; returns (outputs_tuple, BassKernelResults)."""
    in_maps = _host_prep(**{k: np.asarray(v) for k, v in inputs.items()})
    nc = _get_nc()
    res = run_bass_kernel_spmd(nc, in_maps, list(range(NCORES)), trace=trace, **kw)
    r_full = np.concatenate([res.results[k]["r_out"] for k in range(NCORES)], axis=0)
    w_full = np.concatenate([res.results[k]["w_out"] for k in range(NCORES)], axis=0)
    return (r_full, w_full), res


def kernel(**inputs):
    out, _ = run(inputs)
    return out

